# revision 10
# baseline (speedup 1.0000x reference)
"""Causal self-attention (B=2, T=2048, C=2048, H=16, D=128) on 8 trn2 cores.

Sharding: tensor-parallel over heads x data-parallel over batch.
Core c handles batch c//4, heads [4*(c%4) .. 4*(c%4)+4). Each core computes
qkv projection for its 4 heads, RoPE, causal attention, and a partial
output projection (its heads' rows of W_proj); the host sums the 4 partials
per batch.

v3 design (all matmuls bf16 -> PSUM f32; ~4e-3 max-rel error):
  - Q^T/K^T/V live in SBUF in bf16 for the whole kernel: no DRAM scratch.
  - Weights are repacked host-side to partition-major so every DMA moves
    contiguous 4KB runs per partition.
  - Phase 1: QKV projection per 512-col t-block; Q,K produced transposed
    (W^T x^T) with RoPE fused into the PSUM evacuation on DVE; V natural
    via x-as-stationary, evacuated to SBUF by ACT.
  - Phase 2 per (q-block, head), S^T orientation, software-pipelined
    depth 3 (S matmuls run ahead of dn/pv so the ACT exp latency is
    hidden). Diagonal tiles only compute the valid column range
    (q_rel >= j*128); causal masking is a multiplicative 0/1 triangular
    [128,128] bf16 mask on DVE applied post-exp to the one mixed tile.
    denominators: ones^T @ P^T on PE; O^T = PV * recip(dn).
  - Phase 3 interleaved per q-block; PSUM evacuated by DVE (ACT keeps
    doing exp; DMA can't read PSUM); out f32 partials summed on host.
  - PSUM banks: st ring 4 (shared with phase-3 po tiles) + pv 2 + dn 2.
"""

import math
import os

import numpy as np

B, T, C = 2, 2048, 2048
H, D = 16, 128
HPC = 4  # heads per core
NCORES = 8

_CACHE = {}


def _build_program():
    import concourse.tile as tile
    from concourse import bacc, mybir

    f32 = mybir.dt.float32
    bf16 = mybir.dt.bfloat16
    Exp = mybir.ActivationFunctionType.Exp
    SCALE = 1.0 / math.sqrt(float(D))

    nc = bacc.Bacc(
        "TRN2", target_bir_lowering=False, debug=False, num_devices=NCORES
    )

    KT = C // 128  # 16 contraction tiles
    NTB = T // 512  # 4 t-blocks
    MORD = (0, 4, 1, 5, 2, 6, 3, 7)

    # Partition-major packed layouts (see make_in_maps).
    xP = nc.dram_tensor("xP", [128, NTB, KT, 512], bf16, kind="ExternalInput").ap()
    wqkP = nc.dram_tensor("wqkP", [128, 8 * KT * 128], bf16, kind="ExternalInput").ap()
    wvP = nc.dram_tensor("wvP", [128, KT * 512], bf16, kind="ExternalInput").ap()
    wp = nc.dram_tensor("wp", [HPC * D, C], bf16, kind="ExternalInput").ap()
    onesr = nc.dram_tensor("onesr", [128, 128], bf16, kind="ExternalInput").ap()
    cosT = nc.dram_tensor("cosT", [128, T], f32, kind="ExternalInput").ap()
    sinTs = nc.dram_tensor("sinTs", [128, T], f32, kind="ExternalInput").ap()
    mask01 = nc.dram_tensor("mask01", [128, 128], bf16, kind="ExternalInput").ap()
    out = nc.dram_tensor("out", [T, C], bf16, kind="ExternalOutput").ap()

    with tile.TileContext(nc) as tc:
        with (
            tc.tile_pool(name="consts", bufs=1) as consts,
            tc.tile_pool(name="pers", bufs=1) as pers,
        ):
            ones_sb = consts.tile([128, 128], bf16, tag="ones")
            m01_sb = consts.tile([128, 128], bf16, tag="m01")
            cos_sb = consts.tile([128, T], f32, tag="cos")
            sin_sb = consts.tile([128, T], f32, tag="sin")

            qt = [
                pers.tile([128, T], bf16, tag=f"qt{h}", name=f"qt{h}")
                for h in range(HPC)
            ]
            kt = [
                pers.tile([128, T], bf16, tag=f"kt{h}", name=f"kt{h}")
                for h in range(HPC)
            ]
            vt = pers.tile([128, KT, 512], bf16, tag="vt")
            o2 = [
                pers.tile([128, T], bf16, tag=f"o2_{h}", name=f"o2_{h}")
                for h in range(HPC)
            ]
            wps = [
                pers.tile([128, T], bf16, tag=f"wp{i}", name=f"wp{i}")
                for i in range(HPC)
            ]

            # ---------------- Phase 1: QKV projection ----------------
            with (
                tc.tile_pool(name="p1x", bufs=2) as p1x,
                tc.tile_pool(name="p1w", bufs=1) as p1w,
                tc.tile_pool(name="p1e", bufs=2) as p1e,
                tc.tile_pool(name="p1qk", bufs=3, space="PSUM") as p1qk,
                tc.tile_pool(name="p1v", bufs=2, space="PSUM") as p1v,
            ):
                wqkg = p1w.tile([128, 8, KT, 128], bf16, tag="wqkg")
                wv_sb = p1w.tile([128, KT, 512], bf16, tag="wv")

                def load_wm(m):
                    nc.sync.dma_start(
                        out=wqkg[:, m], in_=wqkP[:, m * 2048 : (m + 1) * 2048]
                    )

                def load_x(xtile, tb):
                    for kg in range(4):
                        nc.sync.dma_start(
                            out=xtile[:, kg * 4 : (kg + 1) * 4, :],
                            in_=xP[:, tb, kg * 4 : (kg + 1) * 4, :],
                        )

                # Preamble: what the first m-chain needs goes first; cos/sin
                # are only needed by the (DVE) evacuation, which trails PE.
                nc.sync.dma_start(out=ones_sb, in_=onesr)
                nc.sync.dma_start(out=m01_sb, in_=mask01)
                load_wm(MORD[0])
                xtb0 = p1x.tile([128, KT, 512], bf16, tag="xtb")
                # Stagger x chunk-groups with the later m-weights so the
                # first chains (paced by x arrival) start sooner.
                for kg in range(4):
                    nc.sync.dma_start(
                        out=xtb0[:, kg * 4 : (kg + 1) * 4, :],
                        in_=xP[:, 0, kg * 4 : (kg + 1) * 4, :],
                    )
                    load_wm(MORD[1 + kg])
                nc.sync.dma_start(out=cos_sb[:, 0:512], in_=cosT[:, 0:512])
                nc.sync.dma_start(out=sin_sb[:, 0:512], in_=sinTs[:, 0:512])
                for m in MORD[5:]:
                    load_wm(m)
                for kg in range(4):
                    nc.sync.dma_start(
                        out=wv_sb[:, kg * 4 : (kg + 1) * 4, :],
                        in_=wvP[:, kg * 2048 : (kg + 1) * 2048],
                    )
                xtb1 = p1x.tile([128, KT, 512], bf16, tag="xtb", name="xtb1")
                load_x(xtb1, 1)
                for tbb in range(1, NTB):
                    s = slice(tbb * 512, (tbb + 1) * 512)
                    nc.sync.dma_start(out=cos_sb[:, s], in_=cosT[:, s])
                    nc.sync.dma_start(out=sin_sb[:, s], in_=sinTs[:, s])
                for i in range(HPC):
                    nc.sync.dma_start(out=wps[i], in_=wp[i * 128 : (i + 1) * 128, :])

                xtbs = [xtb0, xtb1, None, None]
                for tb in range(NTB):
                    tsl = slice(tb * 512, (tb + 1) * 512)
                    if tb + 2 < NTB:
                        xn = p1x.tile([128, KT, 512], bf16, tag="xtb",
                                      name=f"xtb{tb + 2}")
                        load_x(xn, tb + 2)
                        xtbs[tb + 2] = xn
                    xtb = xtbs[tb]
                    for m in MORD:
                        ps = p1qk.tile([128, 512], f32, tag="qk")
                        for k in range(KT):
                            nc.tensor.matmul(
                                ps,
                                lhsT=wqkg[:, m, k, :],
                                rhs=xtb[:, k, :],
                                start=(k == 0),
                                stop=(k == KT - 1),
                            )
                        # RoPE fused with PSUM evacuation (DVE), bf16 out.
                        dst = qt[m][:, tsl] if m < 4 else kt[m - 4][:, tsl]
                        tmp = p1e.tile([128, 512], f32, tag="rtmp")
                        nc.vector.tensor_mul(
                            tmp[0:64], ps[64:128], sin_sb[0:64, tsl]
                        )
                        nc.vector.tensor_mul(
                            tmp[64:128], ps[0:64], sin_sb[64:128, tsl]
                        )
                        tmp2 = p1e.tile([128, 512], f32, tag="rtmp2")
                        nc.vector.tensor_mul(tmp2, ps, cos_sb[:, tsl])
                        nc.vector.tensor_add(dst, tmp2, tmp)
                    for tsub in range(4):
                        psv = p1v.tile([128, 512], f32, tag="v")
                        for k in range(KT):
                            nc.tensor.matmul(
                                psv,
                                lhsT=xtb[:, k, tsub * 128 : (tsub + 1) * 128],
                                rhs=wv_sb[:, k, :],
                                start=(k == 0),
                                stop=(k == KT - 1),
                            )
                        nc.scalar.copy(vt[:, tb * 4 + tsub, :], psv)

            # ------------- Phases 2+3 fused per q-block -------------
            with (
                tc.tile_pool(name="p2ps", bufs=1, space="PSUM") as p2ps,
                tc.tile_pool(name="p2pt", bufs=5) as p2pt,
                tc.tile_pool(name="p2s", bufs=2) as p2s,
            ):
                DEPTH = 3
                pending = {}

                def emit_S(qb, h, kb):
                    j = kb - 4 * qb  # >=0 on the diagonal group
                    off = j * 128 if j > 0 else 0
                    st = p2ps.tile([128, 512], f32, tag="st", bufs=4)
                    nc.tensor.matmul(
                        st[:, off:],
                        lhsT=kt[h][:, kb * 128 : (kb + 1) * 128],
                        rhs=qt[h][:, qb * 512 + off : (qb + 1) * 512],
                        start=True,
                        stop=True,
                    )
                    pt = p2pt.tile([128, 512], bf16, tag="pt", bufs=8)
                    nc.scalar.activation(pt[:, off:], st[:, off:], Exp, scale=SCALE)
                    if j >= 0:
                        nc.vector.tensor_mul(
                            pt[:, off : off + 128], pt[:, off : off + 128], m01_sb
                        )
                    pending[(qb, h, kb)] = (pt, off)

                for qb in range(NTB):
                    qsl = slice(qb * 512, (qb + 1) * 512)
                    nk = 4 * (qb + 1)
                    for h in range(HPC):
                        pv = p2ps.tile([128, 512], f32, tag="pv", bufs=2)
                        dn = p2ps.tile([128, 512], f32, tag="dn", bufs=2)
                        for kb in range(min(DEPTH, nk)):
                            if (qb, h, kb) not in pending:
                                emit_S(qb, h, kb)
                        for kb in range(nk):
                            if kb + DEPTH < nk:
                                emit_S(qb, h, kb + DEPTH)
                            pt, off = pending.pop((qb, h, kb))
                            nc.tensor.matmul(
                                dn[:, off:],
                                lhsT=ones_sb,
                                rhs=pt[:, off:],
                                start=(kb == 0),
                                stop=(kb == nk - 1),
                            )
                            nc.tensor.matmul(
                                pv[:, off:],
                                lhsT=vt[:, kb, h * 128 : (h + 1) * 128],
                                rhs=pt[:, off:],
                                start=(kb == 0),
                                stop=(kb == nk - 1),
                            )
                        # dn holds the denominator on every partition.
                        rb = p2s.tile([128, 512], f32, tag="rb")
                        nc.vector.reciprocal_approx_fast(out=rb, in_=dn)
                        nc.vector.tensor_mul(o2[h][:, qsl], pv, rb)
                    # Prime next q-block's first two heads so their exps
                    # overlap this q-block's projection matmuls below and
                    # PE never waits at the q-block boundary.
                    if qb + 1 < NTB:
                        for hh in range(2):
                            for kb in range(DEPTH):
                                emit_S(qb + 1, hh, kb)
                    # Phase 3 for this q-block's four 128-row t-tiles.
                    for tt in range(4):
                        t = qb * 4 + tt
                        tsl = slice(t * 128, (t + 1) * 128)
                        for half in range(2):
                            pos = [
                                p2ps.tile([128, 512], f32, tag="st", bufs=4,
                                          name=f"po{t}_{half}_{i}")
                                for i in range(2)
                            ]
                            for hd in range(HPC):
                                for i in range(2):
                                    cb = half * 2 + i
                                    nc.tensor.matmul(
                                        pos[i],
                                        lhsT=o2[hd][:, tsl],
                                        rhs=wps[hd][:, cb * 512 : (cb + 1) * 512],
                                        start=(hd == 0),
                                        stop=(hd == HPC - 1),
                                    )
                            for i in range(2):
                                cb = half * 2 + i
                                ob = p2s.tile([128, 512], bf16, tag="ob", bufs=4)
                                # Alternate evacuation engine so neither ACT
                                # nor DVE gates the phase-3 PSUM ring.
                                if i == 0:
                                    nc.scalar.copy(ob, pos[i])
                                else:
                                    nc.vector.tensor_copy(ob, pos[i])
                                nc.sync.dma_start(
                                    out=out[tsl, cb * 512 : (cb + 1) * 512],
                                    in_=ob,
                                )
    nc.compile()
    return nc


def _get_program():
    if "nc" not in _CACHE:
        _CACHE["nc"] = _build_program()
    return _CACHE["nc"]


def make_in_maps(x, cos, sin, W_qkv, W_proj):
    """Host-side sharding: per-core input dicts (bf16, partition-major)."""
    import ml_dtypes

    bf = ml_dtypes.bfloat16
    x = np.asarray(x, dtype=np.float32)
    cos = np.asarray(cos, dtype=np.float32)
    sin = np.asarray(sin, dtype=np.float32)
    W_qkv = np.asarray(W_qkv, dtype=np.float32)
    W_proj = np.asarray(W_proj, dtype=np.float32)

    cosT = np.ascontiguousarray(np.tile(cos.T, (2, 1)))  # [128, T]
    sinT = np.ascontiguousarray(np.concatenate([-sin.T, sin.T], axis=0))
    onesr = np.ones((128, 128), dtype=bf)
    # Valid (unmasked) iff q-col >= k-partition within the mixed tile.
    mask01 = np.triu(np.ones((128, 128), dtype=np.float32)).astype(bf)

    in_maps = []
    for core in range(NCORES):
        b, hg = core // 4, core % 4
        csl = slice(hg * 512, (hg + 1) * 512)
        wqk_np = np.concatenate(
            [W_qkv[:, csl], W_qkv[:, C + hg * 512 : C + (hg + 1) * 512]],
            axis=1,
        ).astype(bf)  # [C, 1024]
        # -> [128, (m, k, 128)] partition-major
        wqkP = np.ascontiguousarray(
            wqk_np.reshape(16, 128, 8, 128).transpose(1, 2, 0, 3).reshape(128, -1)
        )
        wv_np = W_qkv[:, 2 * C + hg * 512 : 2 * C + (hg + 1) * 512].astype(bf)
        wvP = np.ascontiguousarray(
            wv_np.reshape(16, 128, 512).transpose(1, 0, 2).reshape(128, -1)
        )
        wp_np = np.ascontiguousarray(W_proj[hg * 512 : (hg + 1) * 512, :]).astype(bf)
        # x[b].T [C, T] -> [128, (tb, k, 512)] partition-major
        xT_np = x[b].T.astype(bf)  # [C, T]
        xPm = np.ascontiguousarray(
            xT_np.reshape(16, 128, 4, 512).transpose(1, 2, 0, 3)
        )  # [128, 4, 16, 512]
        in_maps.append(
            {
                "xP": xPm,
                "wqkP": wqkP,
                "wvP": wvP,
                "wp": wp_np,
                "onesr": onesr,
                "cosT": cosT,
                "sinTs": sinT,
                "mask01": mask01,
            }
        )
    return in_maps


def kernel(x, cos, sin, W_qkv, W_proj):
    from concourse.bass_utils import run_bass_kernel_spmd

    nc = _get_program()
    in_maps = make_in_maps(x, cos, sin, W_qkv, W_proj)
    trace = bool(int(os.environ.get("KERNEL_TRACE", "0")))
    res = run_bass_kernel_spmd(
        nc, in_maps, core_ids=list(range(NCORES)), trace=trace
    )
    if trace:
        _CACHE["last_results"] = res
        if res.exec_time_ns is not None:
            print(f"HW exec time: {res.exec_time_ns} ns")

    out = np.zeros((B, T, C), dtype=np.float32)
    for core in range(NCORES):
        out[core // 4] += np.asarray(res.results[core]["out"], dtype=np.float32)
    return out


# revision 11
# speedup vs baseline: 1.0008x; 1.0008x over previous
"""Causal self-attention (B=2, T=2048, C=2048, H=16, D=128) on 8 trn2 cores.

Sharding: tensor-parallel over heads x data-parallel over batch.
Core c handles batch c//4, heads [4*(c%4) .. 4*(c%4)+4). Each core computes
qkv projection for its 4 heads, RoPE, causal attention, and a partial
output projection (its heads' rows of W_proj); the host sums the 4 partials
per batch.

v3 design (all matmuls bf16 -> PSUM f32; ~4e-3 max-rel error):
  - Q^T/K^T/V live in SBUF in bf16 for the whole kernel: no DRAM scratch.
  - Weights are repacked host-side to partition-major so every DMA moves
    contiguous 4KB runs per partition.
  - Phase 1: QKV projection per 512-col t-block; Q,K produced transposed
    (W^T x^T) with RoPE fused into the PSUM evacuation on DVE; V natural
    via x-as-stationary, evacuated to SBUF by ACT.
  - Phase 2 per (q-block, head), S^T orientation, software-pipelined
    depth 3 (S matmuls run ahead of dn/pv so the ACT exp latency is
    hidden). Diagonal tiles only compute the valid column range
    (q_rel >= j*128); causal masking is a multiplicative 0/1 triangular
    [128,128] bf16 mask on DVE applied post-exp to the one mixed tile.
    denominators: ones^T @ P^T on PE; O^T = PV * recip(dn).
  - Phase 3 interleaved per q-block; PSUM evacuated by DVE (ACT keeps
    doing exp; DMA can't read PSUM); out f32 partials summed on host.
  - PSUM banks: st ring 4 (shared with phase-3 po tiles) + pv 2 + dn 2.
"""

import math
import os

import numpy as np

B, T, C = 2, 2048, 2048
H, D = 16, 128
HPC = 4  # heads per core
NCORES = 8

_CACHE = {}


def _build_program():
    import concourse.tile as tile
    from concourse import bacc, mybir

    f32 = mybir.dt.float32
    bf16 = mybir.dt.bfloat16
    Exp = mybir.ActivationFunctionType.Exp
    SCALE = 1.0 / math.sqrt(float(D))

    nc = bacc.Bacc(
        "TRN2", target_bir_lowering=False, debug=False, num_devices=NCORES
    )

    KT = C // 128  # 16 contraction tiles
    NTB = T // 512  # 4 t-blocks
    MORD = (0, 4, 1, 5, 2, 6, 3, 7)

    # Partition-major packed layouts (see make_in_maps).
    xP = nc.dram_tensor("xP", [128, NTB, KT, 512], bf16, kind="ExternalInput").ap()
    wqkP = nc.dram_tensor("wqkP", [128, 8 * KT * 128], bf16, kind="ExternalInput").ap()
    wvP = nc.dram_tensor("wvP", [128, KT * 512], bf16, kind="ExternalInput").ap()
    wp = nc.dram_tensor("wp", [HPC * D, C], bf16, kind="ExternalInput").ap()
    onesr = nc.dram_tensor("onesr", [128, 128], bf16, kind="ExternalInput").ap()
    cosT = nc.dram_tensor("cosT", [128, T], bf16, kind="ExternalInput").ap()
    sinTs = nc.dram_tensor("sinTs", [128, T], bf16, kind="ExternalInput").ap()
    mask01 = nc.dram_tensor("mask01", [128, 128], bf16, kind="ExternalInput").ap()
    out = nc.dram_tensor("out", [T, C], bf16, kind="ExternalOutput").ap()

    with tile.TileContext(nc) as tc:
        with (
            tc.tile_pool(name="consts", bufs=1) as consts,
            tc.tile_pool(name="pers", bufs=1) as pers,
        ):
            ones_sb = consts.tile([128, 128], bf16, tag="ones")
            m01_sb = consts.tile([128, 128], bf16, tag="m01")
            cos_sb = consts.tile([128, T], bf16, tag="cos")
            sin_sb = consts.tile([128, T], bf16, tag="sin")

            qt = [
                pers.tile([128, T], bf16, tag=f"qt{h}", name=f"qt{h}")
                for h in range(HPC)
            ]
            kt = [
                pers.tile([128, T], bf16, tag=f"kt{h}", name=f"kt{h}")
                for h in range(HPC)
            ]
            vt = pers.tile([128, KT, 512], bf16, tag="vt")
            o2 = [
                pers.tile([128, T], bf16, tag=f"o2_{h}", name=f"o2_{h}")
                for h in range(HPC)
            ]
            wps = [
                pers.tile([128, T], bf16, tag=f"wp{i}", name=f"wp{i}")
                for i in range(HPC)
            ]

            # ---------------- Phase 1: QKV projection ----------------
            with (
                tc.tile_pool(name="p1x", bufs=2) as p1x,
                tc.tile_pool(name="p1w", bufs=1) as p1w,
                tc.tile_pool(name="p1e", bufs=2) as p1e,
                tc.tile_pool(name="p1qk", bufs=4, space="PSUM") as p1qk,
                tc.tile_pool(name="p1v", bufs=2, space="PSUM") as p1v,
            ):
                wqkg = p1w.tile([128, 8, KT, 128], bf16, tag="wqkg")
                wv_sb = p1w.tile([128, KT, 512], bf16, tag="wv")

                def load_wm(m):
                    nc.sync.dma_start(
                        out=wqkg[:, m], in_=wqkP[:, m * 2048 : (m + 1) * 2048]
                    )

                def load_x(xtile, tb):
                    for kg in range(4):
                        nc.sync.dma_start(
                            out=xtile[:, kg * 4 : (kg + 1) * 4, :],
                            in_=xP[:, tb, kg * 4 : (kg + 1) * 4, :],
                        )

                # Preamble: what the first m-chain needs goes first; cos/sin
                # are only needed by the (DVE) evacuation, which trails PE.
                nc.sync.dma_start(out=ones_sb, in_=onesr)
                nc.sync.dma_start(out=m01_sb, in_=mask01)
                load_wm(MORD[0])
                xtb0 = p1x.tile([128, KT, 512], bf16, tag="xtb")
                # Stagger fine x chunk-groups with the later m-weights so
                # the first chain (paced by x arrival) starts sooner.
                for kg in range(8):
                    nc.sync.dma_start(
                        out=xtb0[:, kg * 2 : (kg + 1) * 2, :],
                        in_=xP[:, 0, kg * 2 : (kg + 1) * 2, :],
                    )
                    if kg % 2 == 1 and 1 + kg // 2 < 8:
                        load_wm(MORD[1 + kg // 2])
                nc.sync.dma_start(out=cos_sb[:, 0:512], in_=cosT[:, 0:512])
                nc.sync.dma_start(out=sin_sb[:, 0:512], in_=sinTs[:, 0:512])
                for m in MORD[5:]:
                    load_wm(m)
                for kg in range(4):
                    nc.sync.dma_start(
                        out=wv_sb[:, kg * 4 : (kg + 1) * 4, :],
                        in_=wvP[:, kg * 2048 : (kg + 1) * 2048],
                    )
                xtb1 = p1x.tile([128, KT, 512], bf16, tag="xtb", name="xtb1")
                load_x(xtb1, 1)
                for tbb in range(1, NTB):
                    s = slice(tbb * 512, (tbb + 1) * 512)
                    nc.sync.dma_start(out=cos_sb[:, s], in_=cosT[:, s])
                    nc.sync.dma_start(out=sin_sb[:, s], in_=sinTs[:, s])
                for i in range(HPC):
                    nc.sync.dma_start(out=wps[i], in_=wp[i * 128 : (i + 1) * 128, :])

                xtbs = [xtb0, xtb1, None, None]
                for tb in range(NTB):
                    tsl = slice(tb * 512, (tb + 1) * 512)
                    if tb + 2 < NTB:
                        xn = p1x.tile([128, KT, 512], bf16, tag="xtb",
                                      name=f"xtb{tb + 2}")
                        load_x(xn, tb + 2)
                        xtbs[tb + 2] = xn
                    xtb = xtbs[tb]
                    for m in MORD:
                        ps = p1qk.tile([128, 512], f32, tag="qk")
                        for k in range(KT):
                            nc.tensor.matmul(
                                ps,
                                lhsT=wqkg[:, m, k, :],
                                rhs=xtb[:, k, :],
                                start=(k == 0),
                                stop=(k == KT - 1),
                            )
                        # RoPE fused with PSUM evacuation (DVE), bf16 out.
                        dst = qt[m][:, tsl] if m < 4 else kt[m - 4][:, tsl]
                        tmp = p1e.tile([128, 512], f32, tag="rtmp")
                        nc.vector.tensor_mul(
                            tmp[0:64], ps[64:128], sin_sb[0:64, tsl]
                        )
                        nc.vector.tensor_mul(
                            tmp[64:128], ps[0:64], sin_sb[64:128, tsl]
                        )
                        tmp2 = p1e.tile([128, 512], f32, tag="rtmp2")
                        nc.vector.tensor_mul(tmp2, ps, cos_sb[:, tsl])
                        nc.vector.tensor_add(dst, tmp2, tmp)
                    for tsub in range(4):
                        psv = p1v.tile([128, 512], f32, tag="v")
                        for k in range(KT):
                            nc.tensor.matmul(
                                psv,
                                lhsT=xtb[:, k, tsub * 128 : (tsub + 1) * 128],
                                rhs=wv_sb[:, k, :],
                                start=(k == 0),
                                stop=(k == KT - 1),
                            )
                        nc.scalar.copy(vt[:, tb * 4 + tsub, :], psv)

            # ------------- Phases 2+3 fused per q-block -------------
            with (
                tc.tile_pool(name="p2ps", bufs=1, space="PSUM") as p2ps,
                tc.tile_pool(name="p2pt", bufs=5) as p2pt,
                tc.tile_pool(name="p2s", bufs=2) as p2s,
            ):
                DEPTH = 3
                pending = {}

                def emit_S(qb, h, kb):
                    j = kb - 4 * qb  # >=0 on the diagonal group
                    off = j * 128 if j > 0 else 0
                    st = p2ps.tile([128, 512], f32, tag="st", bufs=4)
                    nc.tensor.matmul(
                        st[:, off:],
                        lhsT=kt[h][:, kb * 128 : (kb + 1) * 128],
                        rhs=qt[h][:, qb * 512 + off : (qb + 1) * 512],
                        start=True,
                        stop=True,
                    )
                    pt = p2pt.tile([128, 512], bf16, tag="pt", bufs=8)
                    nc.scalar.activation(pt[:, off:], st[:, off:], Exp, scale=SCALE)
                    if j >= 0:
                        nc.vector.tensor_mul(
                            pt[:, off : off + 128], pt[:, off : off + 128], m01_sb
                        )
                    pending[(qb, h, kb)] = (pt, off)

                for qb in range(NTB):
                    qsl = slice(qb * 512, (qb + 1) * 512)
                    nk = 4 * (qb + 1)
                    for h in range(HPC):
                        pv = p2ps.tile([128, 512], f32, tag="pv", bufs=2)
                        dn = p2ps.tile([128, 512], f32, tag="dn", bufs=2)
                        for kb in range(min(DEPTH, nk)):
                            if (qb, h, kb) not in pending:
                                emit_S(qb, h, kb)
                        for kb in range(nk):
                            if kb + DEPTH < nk:
                                emit_S(qb, h, kb + DEPTH)
                            pt, off = pending.pop((qb, h, kb))
                            nc.tensor.matmul(
                                dn[:, off:],
                                lhsT=ones_sb,
                                rhs=pt[:, off:],
                                start=(kb == 0),
                                stop=(kb == nk - 1),
                            )
                            nc.tensor.matmul(
                                pv[:, off:],
                                lhsT=vt[:, kb, h * 128 : (h + 1) * 128],
                                rhs=pt[:, off:],
                                start=(kb == 0),
                                stop=(kb == nk - 1),
                            )
                        # dn holds the denominator on every partition.
                        rb = p2s.tile([128, 512], f32, tag="rb")
                        nc.vector.reciprocal_approx_fast(out=rb, in_=dn)
                        nc.vector.tensor_mul(o2[h][:, qsl], pv, rb)
                    # Prime next q-block's first two heads so their exps
                    # overlap this q-block's projection matmuls below and
                    # PE never waits at the q-block boundary.
                    if qb + 1 < NTB:
                        for hh in range(2):
                            for kb in range(DEPTH):
                                emit_S(qb + 1, hh, kb)
                    # Phase 3 for this q-block's four 128-row t-tiles.
                    for tt in range(4):
                        t = qb * 4 + tt
                        tsl = slice(t * 128, (t + 1) * 128)
                        for half in range(2):
                            pos = [
                                p2ps.tile([128, 512], f32, tag="st", bufs=4,
                                          name=f"po{t}_{half}_{i}")
                                for i in range(2)
                            ]
                            for hd in range(HPC):
                                for i in range(2):
                                    cb = half * 2 + i
                                    nc.tensor.matmul(
                                        pos[i],
                                        lhsT=o2[hd][:, tsl],
                                        rhs=wps[hd][:, cb * 512 : (cb + 1) * 512],
                                        start=(hd == 0),
                                        stop=(hd == HPC - 1),
                                    )
                            for i in range(2):
                                cb = half * 2 + i
                                ob = p2s.tile([128, 512], bf16, tag="ob", bufs=4)
                                # Alternate evacuation engine so neither ACT
                                # nor DVE gates the phase-3 PSUM ring.
                                if i == 0:
                                    nc.scalar.copy(ob, pos[i])
                                else:
                                    nc.vector.tensor_copy(ob, pos[i])
                                nc.sync.dma_start(
                                    out=out[tsl, cb * 512 : (cb + 1) * 512],
                                    in_=ob,
                                )
    nc.compile()
    return nc


def _get_program():
    if "nc" not in _CACHE:
        _CACHE["nc"] = _build_program()
    return _CACHE["nc"]


def make_in_maps(x, cos, sin, W_qkv, W_proj):
    """Host-side sharding: per-core input dicts (bf16, partition-major)."""
    import ml_dtypes

    bf = ml_dtypes.bfloat16
    x = np.asarray(x, dtype=np.float32)
    cos = np.asarray(cos, dtype=np.float32)
    sin = np.asarray(sin, dtype=np.float32)
    W_qkv = np.asarray(W_qkv, dtype=np.float32)
    W_proj = np.asarray(W_proj, dtype=np.float32)

    cosT = np.ascontiguousarray(np.tile(cos.T, (2, 1))).astype(bf)  # [128, T]
    sinT = np.ascontiguousarray(np.concatenate([-sin.T, sin.T], axis=0)).astype(bf)
    onesr = np.ones((128, 128), dtype=bf)
    # Valid (unmasked) iff q-col >= k-partition within the mixed tile.
    mask01 = np.triu(np.ones((128, 128), dtype=np.float32)).astype(bf)

    in_maps = []
    for core in range(NCORES):
        b, hg = core // 4, core % 4
        csl = slice(hg * 512, (hg + 1) * 512)
        wqk_np = np.concatenate(
            [W_qkv[:, csl], W_qkv[:, C + hg * 512 : C + (hg + 1) * 512]],
            axis=1,
        ).astype(bf)  # [C, 1024]
        # -> [128, (m, k, 128)] partition-major
        wqkP = np.ascontiguousarray(
            wqk_np.reshape(16, 128, 8, 128).transpose(1, 2, 0, 3).reshape(128, -1)
        )
        wv_np = W_qkv[:, 2 * C + hg * 512 : 2 * C + (hg + 1) * 512].astype(bf)
        wvP = np.ascontiguousarray(
            wv_np.reshape(16, 128, 512).transpose(1, 0, 2).reshape(128, -1)
        )
        wp_np = np.ascontiguousarray(W_proj[hg * 512 : (hg + 1) * 512, :]).astype(bf)
        # x[b].T [C, T] -> [128, (tb, k, 512)] partition-major
        xT_np = x[b].T.astype(bf)  # [C, T]
        xPm = np.ascontiguousarray(
            xT_np.reshape(16, 128, 4, 512).transpose(1, 2, 0, 3)
        )  # [128, 4, 16, 512]
        in_maps.append(
            {
                "xP": xPm,
                "wqkP": wqkP,
                "wvP": wvP,
                "wp": wp_np,
                "onesr": onesr,
                "cosT": cosT,
                "sinTs": sinT,
                "mask01": mask01,
            }
        )
    return in_maps


def kernel(x, cos, sin, W_qkv, W_proj):
    from concourse.bass_utils import run_bass_kernel_spmd

    nc = _get_program()
    in_maps = make_in_maps(x, cos, sin, W_qkv, W_proj)
    trace = bool(int(os.environ.get("KERNEL_TRACE", "0")))
    res = run_bass_kernel_spmd(
        nc, in_maps, core_ids=list(range(NCORES)), trace=trace
    )
    if trace:
        _CACHE["last_results"] = res
        if res.exec_time_ns is not None:
            print(f"HW exec time: {res.exec_time_ns} ns")

    out = np.zeros((B, T, C), dtype=np.float32)
    for core in range(NCORES):
        out[core // 4] += np.asarray(res.results[core]["out"], dtype=np.float32)
    return out


# revision 13
# speedup vs baseline: 1.0034x; 1.0027x over previous
"""Causal self-attention (B=2, T=2048, C=2048, H=16, D=128) on 8 trn2 cores.

Sharding: tensor-parallel over heads x data-parallel over batch.
Core c handles batch c//4, heads [4*(c%4) .. 4*(c%4)+4). Each core computes
qkv projection for its 4 heads, RoPE, causal attention, and a partial
output projection (its heads' rows of W_proj); the host sums the 4 partials
per batch.

v3 design (all matmuls bf16 -> PSUM f32; ~4e-3 max-rel error):
  - Q^T/K^T/V live in SBUF in bf16 for the whole kernel: no DRAM scratch.
  - Weights are repacked host-side to partition-major so every DMA moves
    contiguous 4KB runs per partition.
  - Phase 1: QKV projection per 512-col t-block; Q,K produced transposed
    (W^T x^T) with RoPE fused into the PSUM evacuation on DVE; V natural
    via x-as-stationary, evacuated to SBUF by ACT.
  - Phase 2 per (q-block, head), S^T orientation, software-pipelined
    depth 3 (S matmuls run ahead of dn/pv so the ACT exp latency is
    hidden). Diagonal tiles only compute the valid column range
    (q_rel >= j*128); causal masking is a multiplicative 0/1 triangular
    [128,128] bf16 mask on DVE applied post-exp to the one mixed tile.
    denominators: ones^T @ P^T on PE; O^T = PV * recip(dn).
  - Phase 3 interleaved per q-block; PSUM evacuated by DVE (ACT keeps
    doing exp; DMA can't read PSUM); out f32 partials summed on host.
  - PSUM banks: st ring 4 (shared with phase-3 po tiles) + pv 2 + dn 2.
"""

import math
import os

import numpy as np

B, T, C = 2, 2048, 2048
H, D = 16, 128
HPC = 4  # heads per core
NCORES = 8

_CACHE = {}


def _build_program():
    import concourse.tile as tile
    from concourse import bacc, mybir

    f32 = mybir.dt.float32
    bf16 = mybir.dt.bfloat16
    Exp = mybir.ActivationFunctionType.Exp
    SCALE = 1.0 / math.sqrt(float(D))

    nc = bacc.Bacc(
        "TRN2", target_bir_lowering=False, debug=False, num_devices=NCORES
    )

    KT = C // 128  # 16 contraction tiles
    NTB = T // 512  # 4 t-blocks
    MORD = (0, 4, 1, 5, 2, 6, 3, 7)

    # Partition-major packed layouts (see make_in_maps).
    xP = nc.dram_tensor("xP", [128, NTB, KT, 512], bf16, kind="ExternalInput").ap()
    wqkP = nc.dram_tensor("wqkP", [128, 8 * KT * 128], bf16, kind="ExternalInput").ap()
    wvP = nc.dram_tensor("wvP", [128, KT * 512], bf16, kind="ExternalInput").ap()
    wp = nc.dram_tensor("wp", [HPC * D, C], bf16, kind="ExternalInput").ap()
    onesr = nc.dram_tensor("onesr", [128, 128], bf16, kind="ExternalInput").ap()
    cosT = nc.dram_tensor("cosT", [128, T], bf16, kind="ExternalInput").ap()
    sinTs = nc.dram_tensor("sinTs", [128, T], bf16, kind="ExternalInput").ap()
    mask01 = nc.dram_tensor("mask01", [128, 128], bf16, kind="ExternalInput").ap()
    out = nc.dram_tensor("out", [T, C], bf16, kind="ExternalOutput").ap()

    with tile.TileContext(nc) as tc:
        with (
            tc.tile_pool(name="consts", bufs=1) as consts,
            tc.tile_pool(name="pers", bufs=1) as pers,
        ):
            ones_sb = consts.tile([128, 128], bf16, tag="ones")
            m01_sb = consts.tile([128, 128], bf16, tag="m01")
            cos_sb = consts.tile([128, T], bf16, tag="cos")
            sin_sb = consts.tile([128, T], bf16, tag="sin")

            qt = [
                pers.tile([128, T], bf16, tag=f"qt{h}", name=f"qt{h}")
                for h in range(HPC)
            ]
            kt = [
                pers.tile([128, T], bf16, tag=f"kt{h}", name=f"kt{h}")
                for h in range(HPC)
            ]
            vt = pers.tile([128, KT, 512], bf16, tag="vt")
            o2 = [
                pers.tile([128, T], bf16, tag=f"o2_{h}", name=f"o2_{h}")
                for h in range(HPC)
            ]
            wps = [
                pers.tile([128, T], bf16, tag=f"wp{i}", name=f"wp{i}")
                for i in range(HPC)
            ]

            # ---------------- Phase 1: QKV projection ----------------
            with (
                tc.tile_pool(name="p1x", bufs=2) as p1x,
                tc.tile_pool(name="p1w", bufs=1) as p1w,
                tc.tile_pool(name="p1e", bufs=2) as p1e,
                tc.tile_pool(name="p1qk", bufs=4, space="PSUM") as p1qk,
                tc.tile_pool(name="p1v", bufs=2, space="PSUM") as p1v,
            ):
                wqkg = p1w.tile([128, 8, KT, 128], bf16, tag="wqkg")
                wv_sb = p1w.tile([128, KT, 512], bf16, tag="wv")

                def load_wm(m):
                    nc.sync.dma_start(
                        out=wqkg[:, m], in_=wqkP[:, m * 2048 : (m + 1) * 2048]
                    )

                def load_x(xtile, tb):
                    for kg in range(4):
                        nc.sync.dma_start(
                            out=xtile[:, kg * 4 : (kg + 1) * 4, :],
                            in_=xP[:, tb, kg * 4 : (kg + 1) * 4, :],
                        )

                # Preamble: what the first m-chain needs goes first; cos/sin
                # are only needed by the (DVE) evacuation, which trails PE.
                nc.sync.dma_start(out=ones_sb, in_=onesr)
                nc.sync.dma_start(out=m01_sb, in_=mask01)
                # First weight tile in 4 sub-chunks: the k=0 matmul only
                # needs the first 512 columns.
                for kq in range(4):
                    nc.sync.dma_start(
                        out=wqkg[:, MORD[0], kq * 4 : (kq + 1) * 4, :],
                        in_=wqkP[:, MORD[0] * 2048 + kq * 512 : MORD[0] * 2048 + (kq + 1) * 512],
                    )
                xtb0 = p1x.tile([128, KT, 512], bf16, tag="xtb")
                # Stagger fine x chunk-groups with the later m-weights so
                # the first chain (paced by x arrival) starts sooner.
                for kg in range(8):
                    nc.sync.dma_start(
                        out=xtb0[:, kg * 2 : (kg + 1) * 2, :],
                        in_=xP[:, 0, kg * 2 : (kg + 1) * 2, :],
                    )
                    if kg % 2 == 1 and 1 + kg // 2 < 8:
                        load_wm(MORD[1 + kg // 2])
                nc.sync.dma_start(out=cos_sb[:, 0:512], in_=cosT[:, 0:512])
                nc.sync.dma_start(out=sin_sb[:, 0:512], in_=sinTs[:, 0:512])
                for m in MORD[5:]:
                    load_wm(m)
                for kg in range(4):
                    nc.sync.dma_start(
                        out=wv_sb[:, kg * 4 : (kg + 1) * 4, :],
                        in_=wvP[:, kg * 2048 : (kg + 1) * 2048],
                    )
                xtb1 = p1x.tile([128, KT, 512], bf16, tag="xtb", name="xtb1")
                load_x(xtb1, 1)
                for tbb in range(1, NTB):
                    s = slice(tbb * 512, (tbb + 1) * 512)
                    nc.sync.dma_start(out=cos_sb[:, s], in_=cosT[:, s])
                    nc.sync.dma_start(out=sin_sb[:, s], in_=sinTs[:, s])
                for i in range(HPC):
                    nc.sync.dma_start(out=wps[i], in_=wp[i * 128 : (i + 1) * 128, :])

                xtbs = [xtb0, xtb1, None, None]
                for tb in range(NTB):
                    tsl = slice(tb * 512, (tb + 1) * 512)
                    if tb + 2 < NTB:
                        xn = p1x.tile([128, KT, 512], bf16, tag="xtb",
                                      name=f"xtb{tb + 2}")
                        load_x(xn, tb + 2)
                        xtbs[tb + 2] = xn
                    xtb = xtbs[tb]
                    for m in MORD:
                        ps = p1qk.tile([128, 512], f32, tag="qk")
                        for k in range(KT):
                            nc.tensor.matmul(
                                ps,
                                lhsT=wqkg[:, m, k, :],
                                rhs=xtb[:, k, :],
                                start=(k == 0),
                                stop=(k == KT - 1),
                            )
                        # RoPE fused with PSUM evacuation (DVE), bf16 out.
                        dst = qt[m][:, tsl] if m < 4 else kt[m - 4][:, tsl]
                        tmp = p1e.tile([128, 512], f32, tag="rtmp")
                        nc.vector.tensor_mul(
                            tmp[0:64], ps[64:128], sin_sb[0:64, tsl]
                        )
                        nc.vector.tensor_mul(
                            tmp[64:128], ps[0:64], sin_sb[64:128, tsl]
                        )
                        tmp2 = p1e.tile([128, 512], f32, tag="rtmp2")
                        nc.vector.tensor_mul(tmp2, ps, cos_sb[:, tsl])
                        nc.vector.tensor_add(dst, tmp2, tmp)
                    for tsub in range(4):
                        psv = p1v.tile([128, 512], f32, tag="v")
                        for k in range(KT):
                            nc.tensor.matmul(
                                psv,
                                lhsT=xtb[:, k, tsub * 128 : (tsub + 1) * 128],
                                rhs=wv_sb[:, k, :],
                                start=(k == 0),
                                stop=(k == KT - 1),
                            )
                        nc.scalar.copy(vt[:, tb * 4 + tsub, :], psv)

            # ------------- Phases 2+3 fused per q-block -------------
            with (
                tc.tile_pool(name="p2ps", bufs=1, space="PSUM") as p2ps,
                tc.tile_pool(name="p2pt", bufs=5) as p2pt,
                tc.tile_pool(name="p2s", bufs=2) as p2s,
            ):
                DEPTH = 3
                pending = {}

                def emit_S(qb, h, kb):
                    j = kb - 4 * qb  # >=0 on the diagonal group
                    off = j * 128 if j > 0 else 0
                    st = p2ps.tile([128, 512], f32, tag="st", bufs=4)
                    nc.tensor.matmul(
                        st[:, off:],
                        lhsT=kt[h][:, kb * 128 : (kb + 1) * 128],
                        rhs=qt[h][:, qb * 512 + off : (qb + 1) * 512],
                        start=True,
                        stop=True,
                    )
                    pt = p2pt.tile([128, 512], bf16, tag="pt", bufs=8)
                    nc.scalar.activation(pt[:, off:], st[:, off:], Exp, scale=SCALE)
                    if j >= 0:
                        nc.vector.tensor_mul(
                            pt[:, off : off + 128], pt[:, off : off + 128], m01_sb
                        )
                    pending[(qb, h, kb)] = (pt, off)

                for qb in range(NTB):
                    qsl = slice(qb * 512, (qb + 1) * 512)
                    nk = 4 * (qb + 1)
                    for h in range(HPC):
                        pv = p2ps.tile([128, 512], f32, tag="pv", bufs=2)
                        dn = p2ps.tile([128, 512], f32, tag="dn", bufs=2)
                        for kb in range(min(DEPTH, nk)):
                            if (qb, h, kb) not in pending:
                                emit_S(qb, h, kb)
                        for kb in range(nk):
                            if kb + DEPTH < nk:
                                emit_S(qb, h, kb + DEPTH)
                            pt, off = pending.pop((qb, h, kb))
                            nc.tensor.matmul(
                                dn[:, off:],
                                lhsT=ones_sb,
                                rhs=pt[:, off:],
                                start=(kb == 0),
                                stop=(kb == nk - 1),
                            )
                            nc.tensor.matmul(
                                pv[:, off:],
                                lhsT=vt[:, kb, h * 128 : (h + 1) * 128],
                                rhs=pt[:, off:],
                                start=(kb == 0),
                                stop=(kb == nk - 1),
                            )
                        # dn holds the denominator on every partition.
                        rb = p2s.tile([128, 512], f32, tag="rb")
                        nc.vector.reciprocal_approx_fast(out=rb, in_=dn)
                        nc.vector.tensor_mul(o2[h][:, qsl], pv, rb)
                    # Prime next q-block's first two heads so their exps
                    # overlap this q-block's projection matmuls below and
                    # PE never waits at the q-block boundary.
                    if qb + 1 < NTB:
                        for hh in range(2):
                            for kb in range(DEPTH):
                                emit_S(qb + 1, hh, kb)
                    # Phase 3 for this q-block's four 128-row t-tiles.
                    for tt in range(4):
                        t = qb * 4 + tt
                        tsl = slice(t * 128, (t + 1) * 128)
                        for half in range(2):
                            pos = [
                                p2ps.tile([128, 512], f32, tag="st", bufs=4,
                                          name=f"po{t}_{half}_{i}")
                                for i in range(2)
                            ]
                            for hd in range(HPC):
                                for i in range(2):
                                    cb = half * 2 + i
                                    nc.tensor.matmul(
                                        pos[i],
                                        lhsT=o2[hd][:, tsl],
                                        rhs=wps[hd][:, cb * 512 : (cb + 1) * 512],
                                        start=(hd == 0),
                                        stop=(hd == HPC - 1),
                                    )
                            for i in range(2):
                                cb = half * 2 + i
                                ob = p2s.tile([128, 512], bf16, tag="ob", bufs=4)
                                # Alternate evacuation engine so neither ACT
                                # nor DVE gates the phase-3 PSUM ring.
                                if i == 0:
                                    nc.scalar.copy(ob, pos[i])
                                else:
                                    nc.vector.tensor_copy(ob, pos[i])
                                nc.sync.dma_start(
                                    out=out[tsl, cb * 512 : (cb + 1) * 512],
                                    in_=ob,
                                )
    nc.compile()
    return nc


def _get_program():
    if "nc" not in _CACHE:
        _CACHE["nc"] = _build_program()
    return _CACHE["nc"]


def make_in_maps(x, cos, sin, W_qkv, W_proj):
    """Host-side sharding: per-core input dicts (bf16, partition-major)."""
    import ml_dtypes

    bf = ml_dtypes.bfloat16
    x = np.asarray(x, dtype=np.float32)
    cos = np.asarray(cos, dtype=np.float32)
    sin = np.asarray(sin, dtype=np.float32)
    W_qkv = np.asarray(W_qkv, dtype=np.float32)
    W_proj = np.asarray(W_proj, dtype=np.float32)

    cosT = np.ascontiguousarray(np.tile(cos.T, (2, 1))).astype(bf)  # [128, T]
    sinT = np.ascontiguousarray(np.concatenate([-sin.T, sin.T], axis=0)).astype(bf)
    onesr = np.ones((128, 128), dtype=bf)
    # Valid (unmasked) iff q-col >= k-partition within the mixed tile.
    mask01 = np.triu(np.ones((128, 128), dtype=np.float32)).astype(bf)

    in_maps = []
    for core in range(NCORES):
        b, hg = core // 4, core % 4
        csl = slice(hg * 512, (hg + 1) * 512)
        wqk_np = np.concatenate(
            [W_qkv[:, csl], W_qkv[:, C + hg * 512 : C + (hg + 1) * 512]],
            axis=1,
        ).astype(bf)  # [C, 1024]
        # -> [128, (m, k, 128)] partition-major
        wqkP = np.ascontiguousarray(
            wqk_np.reshape(16, 128, 8, 128).transpose(1, 2, 0, 3).reshape(128, -1)
        )
        wv_np = W_qkv[:, 2 * C + hg * 512 : 2 * C + (hg + 1) * 512].astype(bf)
        wvP = np.ascontiguousarray(
            wv_np.reshape(16, 128, 512).transpose(1, 0, 2).reshape(128, -1)
        )
        wp_np = np.ascontiguousarray(W_proj[hg * 512 : (hg + 1) * 512, :]).astype(bf)
        # x[b].T [C, T] -> [128, (tb, k, 512)] partition-major
        xT_np = x[b].T.astype(bf)  # [C, T]
        xPm = np.ascontiguousarray(
            xT_np.reshape(16, 128, 4, 512).transpose(1, 2, 0, 3)
        )  # [128, 4, 16, 512]
        in_maps.append(
            {
                "xP": xPm,
                "wqkP": wqkP,
                "wvP": wvP,
                "wp": wp_np,
                "onesr": onesr,
                "cosT": cosT,
                "sinTs": sinT,
                "mask01": mask01,
            }
        )
    return in_maps


def kernel(x, cos, sin, W_qkv, W_proj):
    from concourse.bass_utils import run_bass_kernel_spmd

    nc = _get_program()
    in_maps = make_in_maps(x, cos, sin, W_qkv, W_proj)
    trace = bool(int(os.environ.get("KERNEL_TRACE", "0")))
    res = run_bass_kernel_spmd(
        nc, in_maps, core_ids=list(range(NCORES)), trace=trace
    )
    if trace:
        _CACHE["last_results"] = res
        if res.exec_time_ns is not None:
            print(f"HW exec time: {res.exec_time_ns} ns")

    out = np.zeros((B, T, C), dtype=np.float32)
    for core in range(NCORES):
        out[core // 4] += np.asarray(res.results[core]["out"], dtype=np.float32)
    return out
